# revision 5
# baseline (speedup 1.0000x reference)
"""CenterLoss Trainium2 kernel (8 NeuronCores, SPMD via bass).

Reference computation:
    c_sel  = centers[labels]                          # [B, D] gather
    dist_i = ||f_i - c_sel_i||^2
    total  = sum_i dist_i                             # scalar
    out    = total + log_softmax(feats, axis=1)       # [B, D]

Distribution strategy (data-parallel over batch for the output; the scalar
`total` is computed redundantly on every core so no collective is needed):
  - feats / labels / centers are replicated to all 8 cores.
  - every core gathers all 256 center rows (indirect DMA) and reduces the
    full squared-distance total locally (identical result on all cores).
  - each core additionally receives its own 32-row batch shard `feats_ls`
    and emits `total + log_softmax(feats_ls)` -> out shard [32, 512].
  - host concatenates the 8 output shards. No host-side arithmetic.
"""

import numpy as np

from concourse import bass, mybir
from concourse.bass_utils import run_bass_kernel_spmd

B = 256          # batch
D = 512          # feat dim
C = 100000       # num classes
NCORES = 8
BS = B // NCORES  # 32 rows of output per core
P = 128

F32 = mybir.dt.float32
I32 = mybir.dt.int32


def build_nc(num_classes: int = C) -> bass.Bass:
    nc = bass.Bass()

    feats_ext = nc.declare_dram_parameter("feats", [B, D], F32, isOutput=False)
    labels_ext = nc.declare_dram_parameter("labels", [B], I32, isOutput=False)
    fls_ext = nc.declare_dram_parameter("feats_ls", [BS, D], F32, isOutput=False)
    centers_ext = nc.declare_dram_parameter(
        "centers", [num_classes, D], F32, isOutput=False
    )
    out_ext = nc.declare_dram_parameter("out", [BS, D], F32, isOutput=True)

    from contextlib import ExitStack

    with ExitStack() as ctx:
        ec = ctx.enter_context
        # batch row 2p+n lives at partition p, free block n (contiguous per
        # partition for wide DMA descriptors)
        f_sb = ec(nc.sbuf_tensor("f_sb", [P, 2, D], F32))
        c_sb = ec(nc.sbuf_tensor("c_sb", [P, 2, D], F32))
        lbl_sb = ec(nc.sbuf_tensor("lbl_sb", [P, 2], I32))
        fls_sb = ec(nc.sbuf_tensor("fls_sb", [BS, D], F32))
        diff_sb = ec(nc.sbuf_tensor("diff_sb", [P, 2, D], F32))
        sq_sb = ec(nc.sbuf_tensor("sq_sb", [P, 2, D], F32))
        rowsum_sb = ec(nc.sbuf_tensor("rowsum_sb", [P, 1], F32))
        ones128_sb = ec(nc.sbuf_tensor("ones128_sb", [P, 1], F32))
        ones32_sb = ec(nc.sbuf_tensor("ones32_sb", [1, BS], F32))
        negmax_sb = ec(nc.sbuf_tensor("negmax_sb", [BS, 1], F32))
        e_sb = ec(nc.sbuf_tensor("e_sb", [BS, D], F32))
        expsum_sb = ec(nc.sbuf_tensor("expsum_sb", [BS, 1], F32))
        lnss_sb = ec(nc.sbuf_tensor("lnss_sb", [BS, 1], F32))
        s1_sb = ec(nc.sbuf_tensor("s1_sb", [BS, 1], F32))
        s_sb = ec(nc.sbuf_tensor("s_sb", [BS, 1], F32))
        tot_sb = ec(nc.sbuf_tensor("tot_sb", [1, 1], F32))
        outv_sb = ec(nc.sbuf_tensor("outv_sb", [BS, D], F32))
        tot_ps = ec(nc.psum_tensor("tot_ps", [1, 1], F32))
        b32_ps = ec(nc.psum_tensor("b32_ps", [BS, 1], F32))
        fsem = ec(nc.semaphore("fsem"))      # feats DMA
        lsem = ec(nc.semaphore("lsem"))      # labels DMA
        flssem = ec(nc.semaphore("flssem"))  # feats_ls DMA
        gsem = ec(nc.semaphore("gsem"))      # gather DMAs
        vsem = ec(nc.semaphore("vsem"))      # vector ops
        ssem = ec(nc.semaphore("ssem"))      # scalar ops
        psem = ec(nc.semaphore("psem"))      # PE matmuls
        osem = ec(nc.semaphore("osem"))      # output DMA
        block = ec(nc.Block())
        feats_r = feats_ext[:].rearrange("(p n) d -> p n d", n=2)
        labels_r = labels_ext[:].rearrange("(p n) -> p n", n=2)

        @block.sync
        def _(sync):
            sync.dma_start(out=f_sb[:], in_=feats_r).then_inc(fsem, 16)
            sync.dma_start(out=lbl_sb[:], in_=labels_r).then_inc(lsem, 16)
            sync.dma_start(out=fls_sb[:], in_=fls_ext[:]).then_inc(flssem, 16)
            sync.wait_ge(vsem, 7)
            sync.dma_start(out=out_ext[:], in_=outv_sb[:]).then_inc(osem, 16)
            sync.wait_ge(osem, 16)

        @block.gpsimd
        def _(gpsimd):
            gpsimd.wait_ge(lsem, 16)
            for n in range(2):
                gpsimd.indirect_dma_start(
                    out=c_sb[:, n, :],
                    out_offset=None,
                    in_=centers_ext[:],
                    in_offset=bass.IndirectOffsetOnAxis(
                        ap=lbl_sb[:, n : n + 1], axis=0
                    ),
                ).then_inc(gsem, 16)

        @block.vector
        def _(vector):
            vector.memset(ones128_sb[:], 1.0).then_inc(vsem, 1)  # vsem=1
            vector.memset(ones32_sb[:], 1.0).then_inc(vsem, 1)   # vsem=2
            vector.wait_ge(fsem, 16)
            vector.wait_ge(gsem, 32)
            vector.tensor_tensor(
                out=diff_sb[:], in0=f_sb[:], in1=c_sb[:],
                op=mybir.AluOpType.subtract,
            ).then_inc(vsem, 1)                                  # vsem=3
            vector.wait_ge(flssem, 16)
            vector.tensor_reduce(
                out=negmax_sb[:], in_=fls_sb[:],
                axis=mybir.AxisListType.X, op=mybir.AluOpType.max,
                negate=True,
            ).then_inc(vsem, 1)                                  # vsem=4
            # s = (ln(sum exp) + rowmax) - total, then out = fls - s
            vector.wait_ge(ssem, 3)
            vector.wait_ge(vsem, 4)
            vector.tensor_tensor(
                out=s1_sb[:], in0=lnss_sb[:], in1=negmax_sb[:],
                op=mybir.AluOpType.subtract,
            ).then_inc(vsem, 1)                                  # vsem=5
            vector.wait_ge(psem, 2)
            vector.wait_ge(vsem, 5)
            vector.tensor_tensor(
                out=s_sb[:], in0=s1_sb[:], in1=b32_ps[:],
                op=mybir.AluOpType.subtract,
            ).then_inc(vsem, 1)                                  # vsem=6
            vector.wait_ge(vsem, 6)
            vector.tensor_scalar_sub(
                out=outv_sb[:], in0=fls_sb[:], scalar1=s_sb[:, 0:1]
            ).then_inc(vsem, 1)                                  # vsem=7

        @block.scalar
        def _(scalar):
            scalar.wait_ge(vsem, 3)
            scalar.activation(
                out=sq_sb[:], in_=diff_sb[:],
                func=mybir.ActivationFunctionType.Square,
                accum_out=rowsum_sb[:],
            ).then_inc(ssem, 1)                                  # ssem=1
            scalar.wait_ge(vsem, 4)
            scalar.activation(
                out=e_sb[:], in_=fls_sb[:],
                func=mybir.ActivationFunctionType.Exp,
                bias=negmax_sb[:, 0:1],
                accum_out=expsum_sb[:],
            ).then_inc(ssem, 1)                                  # ssem=2
            scalar.wait_ge(ssem, 2)
            scalar.activation(
                out=lnss_sb[:], in_=expsum_sb[:],
                func=mybir.ActivationFunctionType.Ln,
            ).then_inc(ssem, 1)                                  # ssem=3
            scalar.wait_ge(psem, 1)
            scalar.copy(out=tot_sb[:], in_=tot_ps[:]).then_inc(ssem, 1)  # ssem=4

        @block.tensor
        def _(tensor):
            tensor.wait_ge(ssem, 1)
            tensor.wait_ge(vsem, 1)
            # total = sum_p rowsum[p]  -> [1,1] PSUM
            tensor.matmul(
                tot_ps[:], lhsT=rowsum_sb[:], rhs=ones128_sb[:],
                start=True, stop=True,
            ).then_inc(psem, 1)                                  # psem=1
            tensor.wait_ge(ssem, 4)
            tensor.wait_ge(vsem, 2)
            # broadcast total to [BS,1] PSUM
            tensor.matmul(
                b32_ps[:], lhsT=ones32_sb[:], rhs=tot_sb[:],
                start=True, stop=True,
            ).then_inc(psem, 1)                                  # psem=2

    return nc


# test-harness knobs (the grading path leaves these at their defaults)
TRACE = False
_RUN_KWARGS: dict = {}
LAST_RESULT = None

_NC_CACHE: dict[int, bass.Bass] = {}


def _get_nc(num_classes: int) -> bass.Bass:
    if num_classes not in _NC_CACHE:
        _NC_CACHE[num_classes] = build_nc(num_classes)
    return _NC_CACHE[num_classes]


def kernel(feats: np.ndarray, centers: np.ndarray, labels: np.ndarray) -> np.ndarray:
    feats = np.ascontiguousarray(np.asarray(feats, dtype=np.float32))
    centers = np.ascontiguousarray(np.asarray(centers, dtype=np.float32))
    labels_i32 = np.ascontiguousarray(np.asarray(labels).astype(np.int32))
    assert feats.shape == (B, D) and centers.shape[1] == D
    assert labels_i32.shape == (B,)

    nc = _get_nc(centers.shape[0])

    in_maps = [
        {
            "feats": feats,
            "labels": labels_i32,
            "feats_ls": feats[m * BS : (m + 1) * BS],
            "centers": centers,
        }
        for m in range(NCORES)
    ]
    res = run_bass_kernel_spmd(
        nc, in_maps, core_ids=list(range(NCORES)), trace=TRACE, **_RUN_KWARGS
    )
    global LAST_RESULT
    LAST_RESULT = res
    out = np.concatenate([res.results[m]["out"] for m in range(NCORES)], axis=0)
    return out


# revision 7
# speedup vs baseline: 1.1423x; 1.1423x over previous
"""CenterLoss Trainium2 kernel (8 NeuronCores, SPMD via bass).

Reference computation:
    c_sel  = centers[labels]                          # [B, D] gather
    dist_i = ||f_i - c_sel_i||^2
    total  = sum_i dist_i                             # scalar
    out    = total + log_softmax(feats, axis=1)       # [B, D]

Distribution strategy (data-parallel over batch for the output; the scalar
`total` is computed redundantly on every core so no collective is needed):
  - feats / labels / centers are replicated to all 8 cores.
  - every core gathers all 256 center rows (indirect DMA) and reduces the
    full squared-distance total locally (identical result on all cores).
  - each core additionally receives its own 32-row batch shard `feats_ls`
    and emits `total + log_softmax(feats_ls)` -> out shard [32, 512].
  - host concatenates the 8 output shards. No host-side arithmetic.

Latency-oriented layout:
  - labels go first on the sync HWDGE ring (the gather depends on them);
    feats ride the scalar HWDGE ring so the bulk transfer never queues
    ahead of the latency-critical small ones.
  - gather is split in two 128-row tiles so subtract/square pipeline with
    the second tile's DMA flight.
  - total = sum_p rowsum[p] broadcast to the 32 output partitions in one
    step: two accumulating matmuls with a ones [128,32] stationary weight.
  - out = (feats_ls - (ln(sumexp) + rowmax)) + total in a single fused
    tensor_scalar on the vector engine.
"""

import numpy as np

from concourse import bass, mybir
from concourse.bass_utils import run_bass_kernel_spmd

B = 256          # batch
D = 512          # feat dim
C = 100000       # num classes
NCORES = 8
BS = B // NCORES  # 32 rows of output per core
P = 128

F32 = mybir.dt.float32
I32 = mybir.dt.int32


def build_nc(num_classes: int = C) -> bass.Bass:
    nc = bass.Bass()

    feats_ext = nc.declare_dram_parameter("feats", [B, D], F32, isOutput=False)
    labels_ext = nc.declare_dram_parameter("labels", [B], I32, isOutput=False)
    fls_ext = nc.declare_dram_parameter("feats_ls", [BS, D], F32, isOutput=False)
    centers_ext = nc.declare_dram_parameter(
        "centers", [num_classes, D], F32, isOutput=False
    )
    out_ext = nc.declare_dram_parameter("out", [BS, D], F32, isOutput=True)

    from contextlib import ExitStack

    with ExitStack() as ctx:
        ec = ctx.enter_context
        # batch row 2p+n lives at partition p, free block n (contiguous per
        # partition for wide DMA descriptors)
        f_sb = ec(nc.sbuf_tensor("f_sb", [P, 2, D], F32))
        c_sb = ec(nc.sbuf_tensor("c_sb", [P, 2, D], F32))
        lbl_sb = ec(nc.sbuf_tensor("lbl_sb", [P, 2], I32))
        fls_sb = ec(nc.sbuf_tensor("fls_sb", [BS, D], F32))
        u0_sb = ec(nc.sbuf_tensor("u0_sb", [P, D], F32))
        u1_sb = ec(nc.sbuf_tensor("u1_sb", [P, D], F32))
        sq0_sb = ec(nc.sbuf_tensor("sq0_sb", [P, D], F32))
        sq1_sb = ec(nc.sbuf_tensor("sq1_sb", [P, D], F32))
        row0_sb = ec(nc.sbuf_tensor("row0_sb", [P, 1], F32))
        row1_sb = ec(nc.sbuf_tensor("row1_sb", [P, 1], F32))
        onesw_sb = ec(nc.sbuf_tensor("onesw_sb", [P, BS], F32))
        negmax_sb = ec(nc.sbuf_tensor("negmax_sb", [BS, 1], F32))
        e_sb = ec(nc.sbuf_tensor("e_sb", [BS, D], F32))
        expsum_sb = ec(nc.sbuf_tensor("expsum_sb", [BS, 1], F32))
        lnss_sb = ec(nc.sbuf_tensor("lnss_sb", [BS, 1], F32))
        s1_sb = ec(nc.sbuf_tensor("s1_sb", [BS, 1], F32))
        warm_sb = ec(nc.sbuf_tensor("warm_sb", [1, 1], F32))
        outv_sb = ec(nc.sbuf_tensor("outv_sb", [BS, D], F32))
        b32_ps = ec(nc.psum_tensor("b32_ps", [BS, 1], F32))
        fsem = ec(nc.semaphore("fsem"))      # feats DMA
        lsem = ec(nc.semaphore("lsem"))      # labels DMA
        flssem = ec(nc.semaphore("flssem"))  # feats_ls DMA
        g0sem = ec(nc.semaphore("g0sem"))    # gather tile0 DMA
        g1sem = ec(nc.semaphore("g1sem"))    # gather tile1 DMA
        vsem = ec(nc.semaphore("vsem"))      # vector ops
        ssem = ec(nc.semaphore("ssem"))      # scalar ops
        psem = ec(nc.semaphore("psem"))      # PE matmuls
        osem = ec(nc.semaphore("osem"))      # output DMA
        block = ec(nc.Block())

        feats_r = feats_ext[:].rearrange("(p n) d -> p n d", n=2)
        labels_r = labels_ext[:].rearrange("(p n) -> p n", n=2)
        const_one = nc.const_aps.aps[(F32, 1.0)]

        @block.sync
        def _(sync):
            # latency-critical small DMAs only on this ring, labels first
            sync.dma_start(out=lbl_sb[:], in_=labels_r).then_inc(lsem, 16)
            sync.dma_start(out=fls_sb[:], in_=fls_ext[:]).then_inc(flssem, 16)
            sync.wait_ge(vsem, 6)
            sync.dma_start(out=out_ext[:], in_=outv_sb[:]).then_inc(osem, 16)
            sync.wait_ge(osem, 16)

        @block.gpsimd
        def _(gpsimd):
            gpsimd.wait_ge(lsem, 16)
            for n, gnsem in ((0, g0sem), (1, g1sem)):
                gpsimd.indirect_dma_start(
                    out=c_sb[:, n, :],
                    out_offset=None,
                    in_=centers_ext[:],
                    in_offset=bass.IndirectOffsetOnAxis(
                        ap=lbl_sb[:, n : n + 1], axis=0
                    ),
                ).then_inc(gnsem, 16)

        @block.vector
        def _(vector):
            vector.memset(onesw_sb[:], 1.0).then_inc(vsem, 1)    # vsem=1
            vector.wait_ge(flssem, 16)
            vector.tensor_reduce(
                out=negmax_sb[:], in_=fls_sb[:],
                axis=mybir.AxisListType.X, op=mybir.AluOpType.max,
                negate=True,
            ).then_inc(vsem, 1)                                  # vsem=2
            vector.wait_ge(fsem, 16)
            vector.wait_ge(g0sem, 16)
            vector.tensor_tensor(
                out=u0_sb[:], in0=f_sb[:, 0, :], in1=c_sb[:, 0, :],
                op=mybir.AluOpType.subtract,
            ).then_inc(vsem, 1)                                  # vsem=3
            vector.wait_ge(g1sem, 16)
            vector.tensor_tensor(
                out=u1_sb[:], in0=f_sb[:, 1, :], in1=c_sb[:, 1, :],
                op=mybir.AluOpType.subtract,
            ).then_inc(vsem, 1)                                  # vsem=4
            # s1 = ln(sum exp) + rowmax
            vector.wait_ge(ssem, 2)
            vector.wait_ge(vsem, 2)
            vector.tensor_tensor(
                out=s1_sb[:], in0=lnss_sb[:], in1=negmax_sb[:],
                op=mybir.AluOpType.subtract,
            ).then_inc(vsem, 1)                                  # vsem=5
            # out = (fls - s1) + total
            vector.wait_ge(psem, 2)
            vector.wait_ge(vsem, 5)
            vector.tensor_scalar(
                out=outv_sb[:], in0=fls_sb[:],
                scalar1=s1_sb[:, 0:1], scalar2=b32_ps[:, 0:1],
                op0=mybir.AluOpType.subtract, op1=mybir.AluOpType.add,
            ).then_inc(vsem, 1)                                  # vsem=6

        @block.scalar
        def _(scalar):
            # bulk feats transfer on the scalar HWDGE ring
            scalar.dma_start(out=f_sb[:], in_=feats_r).then_inc(fsem, 16)
            # warm the activation table off the critical path
            scalar.activation(
                out=warm_sb[:], in_=const_one[0:1, 0:1],
                func=mybir.ActivationFunctionType.Square,
            )
            scalar.wait_ge(vsem, 2)
            scalar.activation(
                out=e_sb[:], in_=fls_sb[:],
                func=mybir.ActivationFunctionType.Exp,
                bias=negmax_sb[:, 0:1],
                accum_out=expsum_sb[:],
            ).then_inc(ssem, 1)                                  # ssem=1
            scalar.wait_ge(ssem, 1)
            scalar.activation(
                out=lnss_sb[:], in_=expsum_sb[:],
                func=mybir.ActivationFunctionType.Ln,
            ).then_inc(ssem, 1)                                  # ssem=2
            scalar.wait_ge(vsem, 3)
            scalar.activation(
                out=sq0_sb[:], in_=u0_sb[:],
                func=mybir.ActivationFunctionType.Square,
                accum_out=row0_sb[:],
            ).then_inc(ssem, 1)                                  # ssem=3
            scalar.wait_ge(vsem, 4)
            scalar.activation(
                out=sq1_sb[:], in_=u1_sb[:],
                func=mybir.ActivationFunctionType.Square,
                accum_out=row1_sb[:],
            ).then_inc(ssem, 1)                                  # ssem=4

        @block.tensor
        def _(tensor):
            # b32 = ones.T @ (row0 + row1): total broadcast to BS partitions
            tensor.wait_ge(ssem, 3)
            tensor.wait_ge(vsem, 1)
            tensor.matmul(
                b32_ps[:], lhsT=onesw_sb[:], rhs=row0_sb[:],
                start=True, stop=False,
            ).then_inc(psem, 1)                                  # psem=1
            tensor.wait_ge(ssem, 4)
            tensor.matmul(
                b32_ps[:], lhsT=onesw_sb[:], rhs=row1_sb[:],
                start=False, stop=True,
            ).then_inc(psem, 1)                                  # psem=2

    return nc


# test-harness knobs (the grading path leaves these at their defaults)
TRACE = False
_RUN_KWARGS: dict = {}
LAST_RESULT = None

_NC_CACHE: dict[int, bass.Bass] = {}


def _get_nc(num_classes: int) -> bass.Bass:
    if num_classes not in _NC_CACHE:
        _NC_CACHE[num_classes] = build_nc(num_classes)
    return _NC_CACHE[num_classes]


def kernel(feats: np.ndarray, centers: np.ndarray, labels: np.ndarray) -> np.ndarray:
    feats = np.ascontiguousarray(np.asarray(feats, dtype=np.float32))
    centers = np.ascontiguousarray(np.asarray(centers, dtype=np.float32))
    labels_i32 = np.ascontiguousarray(np.asarray(labels).astype(np.int32))
    assert feats.shape == (B, D) and centers.shape[1] == D
    assert labels_i32.shape == (B,)

    nc = _get_nc(centers.shape[0])

    in_maps = [
        {
            "feats": feats,
            "labels": labels_i32,
            "feats_ls": feats[m * BS : (m + 1) * BS],
            "centers": centers,
        }
        for m in range(NCORES)
    ]
    res = run_bass_kernel_spmd(
        nc, in_maps, core_ids=list(range(NCORES)), trace=TRACE, **_RUN_KWARGS
    )
    global LAST_RESULT
    LAST_RESULT = res
    out = np.concatenate([res.results[m]["out"] for m in range(NCORES)], axis=0)
    return out


# revision 11
# speedup vs baseline: 1.2087x; 1.0582x over previous
"""CenterLoss Trainium2 kernel (8 NeuronCores, SPMD via bass).

Reference computation:
    c_sel  = centers[labels]                          # [B, D] gather
    dist_i = ||f_i - c_sel_i||^2
    total  = sum_i dist_i                             # scalar
    out    = total + log_softmax(feats, axis=1)       # [B, D]

Distribution strategy (data-parallel over batch for the output; the scalar
`total` is computed redundantly on every core so no collective is needed):
  - feats / labels / centers are replicated to all 8 cores.
  - every core gathers all 256 center rows (indirect DMA) and reduces the
    full squared-distance total locally (identical result on all cores).
  - each core additionally receives its own 32-row batch shard `feats_ls`
    and emits `total + log_softmax(feats_ls)` -> out shard [32, 512].
  - host concatenates the 8 output shards. No host-side arithmetic.

Latency-oriented layout:
  - batch row 2p+n lives at partition p, free block n: feats is a
    contiguous 4KB-per-partition DMA and labels an 8B-per-partition DMA.
  - labels ride the sync HWDGE ring first (the gather depends on them);
    feats ride the scalar HWDGE ring so the bulk transfer never queues
    ahead of the latency-critical small ones.
  - the gather is split in two 128-row tiles, and subtract/square is
    further split in two 256-wide halves per tile so the vector subtract
    of one half overlaps the scalar square+accumulate of the previous.
  - total = sum_p rowsum[p] lands broadcast on the 32 output partitions
    via four accumulating matmuls with a ones [128,32] stationary weight.
  - out = (feats_ls - (ln(sumexp) + rowmax)) + total in a single fused
    tensor_scalar on the vector engine.
"""

import numpy as np

from concourse import bass, mybir
from concourse.bass_utils import run_bass_kernel_spmd

B = 256          # batch
D = 512          # feat dim
DH = D // 2      # half of the feature dim (pipeline granule)
C = 100000       # num classes
NCORES = 8
BS = B // NCORES  # 32 rows of output per core
P = 128

F32 = mybir.dt.float32
I32 = mybir.dt.int32


def build_nc(num_classes: int = C) -> bass.Bass:
    nc = bass.Bass()

    feats_ext = nc.declare_dram_parameter("feats", [B, D], F32, isOutput=False)
    labels_ext = nc.declare_dram_parameter("labels", [B], I32, isOutput=False)
    fls_ext = nc.declare_dram_parameter("feats_ls", [BS, D], F32, isOutput=False)
    centers_ext = nc.declare_dram_parameter(
        "centers", [num_classes, D], F32, isOutput=False
    )
    out_ext = nc.declare_dram_parameter("out", [BS, D], F32, isOutput=True)

    from contextlib import ExitStack

    with ExitStack() as ctx:
        ec = ctx.enter_context
        f_sb = ec(nc.sbuf_tensor("f_sb", [P, 2, D], F32))
        c_sb = ec(nc.sbuf_tensor("c_sb", [P, 2, D], F32))
        lbl_sb = ec(nc.sbuf_tensor("lbl_sb", [P, 2], I32))
        fls_sb = ec(nc.sbuf_tensor("fls_sb", [BS, D], F32))
        u_sb = ec(nc.sbuf_tensor("u_sb", [P, 2, D], F32))
        sq_sb = ec(nc.sbuf_tensor("sq_sb", [P, 2, D], F32))
        row_sb = ec(nc.sbuf_tensor("row_sb", [P, 4], F32))
        onesw_sb = ec(nc.sbuf_tensor("onesw_sb", [P, BS], F32))
        negmax_sb = ec(nc.sbuf_tensor("negmax_sb", [BS, 1], F32))
        e_sb = ec(nc.sbuf_tensor("e_sb", [BS, D], F32))
        expsum_sb = ec(nc.sbuf_tensor("expsum_sb", [BS, 1], F32))
        lnss_sb = ec(nc.sbuf_tensor("lnss_sb", [BS, 1], F32))
        s1_sb = ec(nc.sbuf_tensor("s1_sb", [BS, 1], F32))
        warm_sb = ec(nc.sbuf_tensor("warm_sb", [1, 1], F32))
        outv_sb = ec(nc.sbuf_tensor("outv_sb", [BS, D], F32))
        b32_ps = ec(nc.psum_tensor("b32_ps", [BS, 1], F32))
        fsem = ec(nc.semaphore("fsem"))      # feats DMA
        lsem = ec(nc.semaphore("lsem"))      # labels DMA
        flssem = ec(nc.semaphore("flssem"))  # feats_ls DMA
        g0sem = ec(nc.semaphore("g0sem"))    # gather tile0 DMA
        g1sem = ec(nc.semaphore("g1sem"))    # gather tile1 DMA
        vsem = ec(nc.semaphore("vsem"))      # vector ops
        ssem = ec(nc.semaphore("ssem"))      # scalar ops
        psem = ec(nc.semaphore("psem"))      # PE matmuls
        osem = ec(nc.semaphore("osem"))      # output DMA
        block = ec(nc.Block(no_gpsimd_drain=True))

        feats_r = feats_ext[:].rearrange("(p n) d -> p n d", n=2)
        labels_r = labels_ext[:].rearrange("(p n) -> p n", n=2)
        const_one = nc.const_aps.aps[(F32, 1.0)]

        # (tile, half) -> chunk index in issue order
        chunks = [(t, h) for t in range(2) for h in range(2)]

        @block.sync
        def _(sync):
            # latency-critical small DMAs only on this ring, labels first
            sync.dma_start(out=lbl_sb[:], in_=labels_r).then_inc(lsem, 16)
            sync.dma_start(out=fls_sb[:], in_=fls_ext[:]).then_inc(flssem, 16)
            sync.wait_ge(vsem, 8)
            sync.dma_start(out=out_ext[:], in_=outv_sb[:]).then_inc(osem, 16)
            sync.wait_ge(osem, 16)

        @block.gpsimd
        def _(gpsimd):
            gpsimd.wait_ge(lsem, 16)
            for n, gnsem in ((0, g0sem), (1, g1sem)):
                gpsimd.indirect_dma_start(
                    out=c_sb[:, n, :],
                    out_offset=None,
                    in_=centers_ext[:],
                    in_offset=bass.IndirectOffsetOnAxis(
                        ap=lbl_sb[:, n : n + 1], axis=0
                    ),
                ).then_inc(gnsem, 16)

        @block.vector
        def _(vector):
            vector.memset(onesw_sb[:], 1.0).then_inc(vsem, 1)    # vsem=1
            vector.wait_ge(flssem, 16)
            vector.tensor_reduce(
                out=negmax_sb[:], in_=fls_sb[:],
                axis=mybir.AxisListType.X, op=mybir.AluOpType.max,
                negate=True,
            ).then_inc(vsem, 1)                                  # vsem=2
            vector.wait_ge(fsem, 16)
            for i, (t, h) in enumerate(chunks):
                vector.wait_ge((g0sem, g1sem)[t], 16)
                hs = slice(h * DH, (h + 1) * DH)
                vector.tensor_tensor(
                    out=u_sb[:, t, hs], in0=f_sb[:, t, hs], in1=c_sb[:, t, hs],
                    op=mybir.AluOpType.subtract,
                ).then_inc(vsem, 1)                              # vsem=3..6
            # s1 = ln(sum exp) + rowmax
            vector.wait_ge(ssem, 2)
            vector.wait_ge(vsem, 2)
            vector.tensor_tensor(
                out=s1_sb[:], in0=lnss_sb[:], in1=negmax_sb[:],
                op=mybir.AluOpType.subtract,
            ).then_inc(vsem, 1)                                  # vsem=7
            # out = (fls - s1) + total
            vector.wait_ge(psem, 4)
            vector.wait_ge(vsem, 7)
            vector.tensor_scalar(
                out=outv_sb[:], in0=fls_sb[:],
                scalar1=s1_sb[:, 0:1], scalar2=b32_ps[:, 0:1],
                op0=mybir.AluOpType.subtract, op1=mybir.AluOpType.add,
            ).then_inc(vsem, 1)                                  # vsem=8

        @block.scalar
        def _(scalar):
            # bulk feats transfer on the scalar HWDGE ring
            scalar.dma_start(out=f_sb[:], in_=feats_r).then_inc(fsem, 16)
            # warm the activation table off the critical path
            scalar.activation(
                out=warm_sb[:], in_=const_one[0:1, 0:1],
                func=mybir.ActivationFunctionType.Square,
            )
            scalar.wait_ge(vsem, 2)
            scalar.activation(
                out=e_sb[:], in_=fls_sb[:],
                func=mybir.ActivationFunctionType.Exp,
                bias=negmax_sb[:, 0:1],
                accum_out=expsum_sb[:],
            ).then_inc(ssem, 1)                                  # ssem=1
            scalar.wait_ge(ssem, 1)
            scalar.activation(
                out=lnss_sb[:], in_=expsum_sb[:],
                func=mybir.ActivationFunctionType.Ln,
            ).then_inc(ssem, 1)                                  # ssem=2
            for i, (t, h) in enumerate(chunks):
                hs = slice(h * DH, (h + 1) * DH)
                scalar.wait_ge(vsem, 3 + i)
                scalar.activation(
                    out=sq_sb[:, t, hs], in_=u_sb[:, t, hs],
                    func=mybir.ActivationFunctionType.Square,
                    accum_out=row_sb[:, i : i + 1],
                ).then_inc(ssem, 1)                              # ssem=3..6

        @block.tensor
        def _(tensor):
            # b32 = ones.T @ sum_i row_i: total broadcast to BS partitions
            tensor.wait_ge(vsem, 1)
            for i in range(4):
                tensor.wait_ge(ssem, 3 + i)
                tensor.matmul(
                    b32_ps[:], lhsT=onesw_sb[:], rhs=row_sb[:, i : i + 1],
                    start=(i == 0), stop=(i == 3),
                ).then_inc(psem, 1)                              # psem=1..4

    return nc


# test-harness knobs (the grading path leaves these at their defaults)
TRACE = False
_RUN_KWARGS: dict = {}
LAST_RESULT = None

_NC_CACHE: dict[int, bass.Bass] = {}


def _get_nc(num_classes: int) -> bass.Bass:
    if num_classes not in _NC_CACHE:
        _NC_CACHE[num_classes] = build_nc(num_classes)
    return _NC_CACHE[num_classes]


def kernel(feats: np.ndarray, centers: np.ndarray, labels: np.ndarray) -> np.ndarray:
    feats = np.ascontiguousarray(np.asarray(feats, dtype=np.float32))
    centers = np.ascontiguousarray(np.asarray(centers, dtype=np.float32))
    labels_i32 = np.ascontiguousarray(np.asarray(labels).astype(np.int32))
    assert feats.shape == (B, D) and centers.shape[1] == D
    assert labels_i32.shape == (B,)

    nc = _get_nc(centers.shape[0])

    in_maps = [
        {
            "feats": feats,
            "labels": labels_i32,
            "feats_ls": feats[m * BS : (m + 1) * BS],
            "centers": centers,
        }
        for m in range(NCORES)
    ]
    res = run_bass_kernel_spmd(
        nc, in_maps, core_ids=list(range(NCORES)), trace=TRACE, **_RUN_KWARGS
    )
    global LAST_RESULT
    LAST_RESULT = res
    out = np.concatenate([res.results[m]["out"] for m in range(NCORES)], axis=0)
    return out
